# revision 2
# baseline (speedup 1.0000x reference)
"""Trainium2 Bass kernel for nn_GroupPointEncoder.

Reference computation (G=4, B=8, N=2048, F=128):
  std = 2 or 4 per point by label class
  coords = [point_coord, (point_coord + noise*std)[1:]]           # [G,B,N,3]
  normed = (coords - low) / (high - low)
  pe     = interleaved sin/cos embedding, (y,x,z) order            # [G,B,N,384]
  h      = relu(pe @ W1.T + b1)                                    # [G,B,N,512]
  pos    = h @ W2.T + b2                                           # [G,B,N,256]
  query  = label_weight[labels] + pos
  out    = concat([query_pos, query], -1).reshape(G*B, N, 512)

Sharding: data-parallel over the G*B=32 (g,b) pairs, 4 per core, 8 cores.
Each core computes its 4*2048=8192 points' `query` half on device; the
query_pos half is a passthrough assembled on the host.

Device pipeline (per 512-point tile, engines balanced under the PE pole):
  PE   : arg_c[128,T](PSUM) = [s; phase]^T @ [xs_c; 1]   3 K=2 matmuls
         h(PSUM)  = W1p @ pe          12 f32r matmuls
         q(PSUM)  = W2 @ h + lwb @ onehot   10 f32r matmuls
  DVE  : argS = copy(arg_c) PSUM->SBUF x3;  q copies PSUM->SBUF x2
  Pool : ki = round(argS/2pi); kf = ki*2pi; red = argS - kf   (range reduce)
  ACT  : pe = Sin(red)  (cos phase pre-added in the matmul);  relu x4
Two-tile software pipelining keeps the PE warm (HAM at 2.4 GHz).
"""
import sys
import math

sys.path.insert(0, "/opt/trn_rl_repo")

import numpy as np
from contextlib import ExitStack

import concourse.bass as bass
import concourse.tile as tile
from concourse import bacc, library_config, mybir
from concourse.bass_utils import run_bass_kernel_spmd

# problem constants (hardcoded per contract)
G, B, N, F = 4, 8, 2048, 128
NCORES = 8
BPC = B * G // NCORES          # 4 (g,b) pairs per core
NPTS = BPC * N                 # 8192 points per core
T = 512                        # points per tile
NT = NPTS // T                 # 16 tiles
TWO_PI = 2.0 * math.pi
INV_TWO_PI = 1.0 / TWO_PI
F32 = mybir.dt.float32
F32R = mybir.dt.float32r
I32 = mybir.dt.int32

_CACHE = {}


def _build_program():
    nc = bacc.Bacc("TRN2", target_bir_lowering=False, debug=False, num_devices=NCORES)

    xs_d = nc.dram_tensor("xs", [NT, 2, 3, T], F32R, kind="ExternalInput").ap()
    oh_d = nc.dram_tensor("oh", [NT, 10, T], F32R, kind="ExternalInput").ap()
    st_d = nc.dram_tensor("st", [2, 128], F32R, kind="ExternalInput").ap()
    w1t_d = nc.dram_tensor("w1t", [3, 128, 512], F32R, kind="ExternalInput").ap()
    w2t_d = nc.dram_tensor("w2t", [4, 128, 256], F32R, kind="ExternalInput").ap()
    lwb_d = nc.dram_tensor("lwb", [10, 256], F32R, kind="ExternalInput").ap()
    b1c_d = nc.dram_tensor("b1c", [128, 4], F32, kind="ExternalInput").ap()
    q_d = nc.dram_tensor("q", [256, NPTS], F32, kind="ExternalOutput").ap()

    with tile.TileContext(nc) as tc, ExitStack() as ctx:
        cpool = ctx.enter_context(tc.tile_pool(name="consts", bufs=1))
        wpool = ctx.enter_context(tc.tile_pool(name="weights", bufs=1))
        io = ctx.enter_context(tc.tile_pool(name="io", bufs=4))
        argp = ctx.enter_context(tc.tile_pool(name="argS", bufs=2))
        redp = ctx.enter_context(tc.tile_pool(name="red", bufs=2))
        pep = ctx.enter_context(tc.tile_pool(name="pe", bufs=3))
        hw = ctx.enter_context(tc.tile_pool(name="hwork", bufs=2))
        qw = ctx.enter_context(tc.tile_pool(name="qwork", bufs=2))
        psum_a = ctx.enter_context(tc.tile_pool(name="pa", bufs=2, space="PSUM"))
        psum_h = ctx.enter_context(tc.tile_pool(name="ph", bufs=1, space="PSUM"))
        psum_q = ctx.enter_context(tc.tile_pool(name="pq", bufs=1, space="PSUM"))

        nc.gpsimd.load_library(library_config.proxy)
        st = cpool.tile([2, 128], F32R)
        nc.sync.dma_start(st[:], st_d[:])
        b1c = cpool.tile([128, 4], F32)
        nc.sync.dma_start(b1c[:], b1c_d[:])
        lwb = cpool.tile([10, 256], F32R)
        nc.sync.dma_start(lwb[:], lwb_d[:])

        w1t = []
        for k in range(3):
            w = wpool.tile([128, 512], F32R, name=f"w1t{k}", tag=f"w1t{k}")
            nc.sync.dma_start(w[:], w1t_d[k])
            w1t.append(w)
        w2t = []
        for k in range(4):
            w = wpool.tile([128, 256], F32R, name=f"w2t{k}", tag=f"w2t{k}")
            nc.sync.dma_start(w[:], w2t_d[k])
            w2t.append(w)

        # software pipeline state: produce index t, compute index u = t-2
        xs_t = [None] * NT
        oh_t = [None] * NT
        pe_t = [None] * NT

        for i in range(NT + 2):
            t = i
            u = i - 2

            if t < NT:
                xs = io.tile([2, 3, T], F32R, tag="xs")
                nc.sync.dma_start(xs[:], xs_d[t])
                oh = io.tile([10, T], F32R, tag="oh")
                nc.sync.dma_start(oh[:], oh_d[t])
                xs_t[t], oh_t[t] = xs, oh

                # arg_c = s*x + phase, K=2 outer product into PSUM
                argS = argp.tile([128, 3, T], F32, tag="argS")
                a0 = psum_a.tile([128, T], F32, tag="arg")
                nc.tensor.matmul(a0[:], st[:], xs[:, 0, :], start=True, stop=True)
                a1 = psum_a.tile([128, T], F32, tag="arg")
                nc.tensor.matmul(a1[:], st[:], xs[:, 1, :], start=True, stop=True)
                nc.vector.tensor_copy(argS[:, 0, :], a0[:])
                nc.vector.tensor_copy(argS[:, 1, :], a1[:])

            if u >= 0:
                # stage 4 (m=0): h[0] = relu(W1p[:,0:128] @ pe + b1[0])
                hp = psum_h.tile([128, 4, T], F32, tag="hp")
                h = hw.tile([128, 4, T], F32R, tag="h")
                pe = pe_t[u]
                for k in range(3):
                    nc.tensor.matmul(
                        hp[:, 0, :], w1t[k][:, 0:128], pe[:, k, :],
                        start=(k == 0), stop=(k == 2),
                    )
                nc.scalar.activation(
                    h[:, 0, :], hp[:, 0, :],
                    mybir.ActivationFunctionType.Relu, bias=b1c[:, 0:1],
                )

            if t < NT:
                a2 = psum_a.tile([128, T], F32, tag="arg")
                nc.tensor.matmul(a2[:], st[:], xs_t[t][:, 2, :], start=True, stop=True)
                nc.vector.tensor_copy(argS[:, 2, :], a2[:])

            if u >= 0:
                for m in range(1, 4):
                    for k in range(3):
                        nc.tensor.matmul(
                            hp[:, m, :], w1t[k][:, m * 128 : (m + 1) * 128], pe[:, k, :],
                            start=(k == 0), stop=(k == 2),
                        )
                    nc.scalar.activation(
                        h[:, m, :], hp[:, m, :],
                        mybir.ActivationFunctionType.Relu, bias=b1c[:, m : m + 1],
                    )

            if t < NT:
                # range reduction on Pool: red = argS - 2pi*round(argS/2pi)
                ki = redp.tile([128, 3, T], I32, tag="ki")
                nc.gpsimd.tensor_scalar(
                    ki[:], argS[:], INV_TWO_PI, None, op0=mybir.AluOpType.mult
                )
                kf = redp.tile([128, 3, T], F32, tag="kf")
                nc.gpsimd.tensor_scalar(
                    kf[:], ki[:], TWO_PI, None, op0=mybir.AluOpType.mult
                )
                red = redp.tile([128, 3, T], F32, tag="red")
                nc.gpsimd.tensor_sub(red[:], argS[:], kf[:])
                peo = pep.tile([128, 3, T], F32R, tag="pe")
                nc.scalar.activation(peo[:], red[:], mybir.ActivationFunctionType.Sin)
                pe_t[t] = peo

            if u >= 0:
                # stage 5: q = W2 @ h + lwb @ onehot (accumulated in PSUM)
                qp = psum_q.tile([128, 2, T], F32, tag="qp")
                qs = qw.tile([128, 2, T], F32, tag="qs")
                for mp in range(2):
                    nc.tensor.matmul(
                        qp[:, mp, :], lwb[:, mp * 128 : (mp + 1) * 128], oh_t[u][:],
                        start=True, stop=False,
                    )
                    for k in range(4):
                        nc.tensor.matmul(
                            qp[:, mp, :], w2t[k][:, mp * 128 : (mp + 1) * 128],
                            h[:, k, :], start=False, stop=(k == 3),
                        )
                    nc.vector.tensor_copy(qs[:, mp, :], qp[:, mp, :])
                    nc.sync.dma_start(
                        q_d[mp * 128 : (mp + 1) * 128, u * T : (u + 1) * T],
                        qs[:, mp, :],
                    )
                xs_t[u] = oh_t[u] = pe_t[u] = None

    nc.compile()
    return nc


def _host_prep(point_coord, labels, pc_range, noise, label_weight, W1, b1, W2, b2):
    """Build the per-core input maps (host-side sharding + weight prep)."""
    pc32 = np.asarray(point_coord, np.float32)
    lab = np.asarray(labels)
    noi = np.asarray(noise, np.float32)
    rng = np.asarray(pc_range, np.float32)

    small = (lab == 0) | (lab >= 6)
    std = np.where(small, 2.0, 4.0).astype(np.float32)            # [B,N]
    coords = pc32[None] + noi * std[None, :, :, None]             # [G,B,N,3]
    coords[0] = pc32                                              # group 0 originals
    low, high = rng[:3], rng[3:]
    pcs = (coords - low) / (high - low) * np.float32(TWO_PI)      # [G,B,N,3]
    pcs = pcs[..., [1, 0, 2]]   # reference concatenates pe in (y,x,z) order
    onehot = np.eye(10, dtype=np.float32)[np.asarray(lab, np.int64)]  # [B,N,10]

    # feature permutation: kernel row c*128+k -> ref feature c*128+2k (sin),
    # row c*128+64+k -> c*128+2k+1 (cos)
    perm = np.empty(3 * F, np.int64)
    for c in range(3):
        for k in range(64):
            perm[c * 128 + k] = c * 128 + 2 * k
            perm[c * 128 + 64 + k] = c * 128 + 2 * k + 1
    w1p = np.ascontiguousarray(np.asarray(W1, np.float32)[:, perm].T)  # [384,512]
    w2t = np.ascontiguousarray(np.asarray(W2, np.float32).T)           # [512,256]
    lwb = np.asarray(label_weight, np.float32) + np.asarray(b2, np.float32)[None]
    b1c = np.ascontiguousarray(np.asarray(b1, np.float32).reshape(4, 128).T)

    k64 = np.arange(64, dtype=np.float64)
    s64 = 10000.0 ** (-k64 / 64.0)
    s128 = np.concatenate([s64, s64])
    st = np.zeros((2, 128), np.float32)
    st[0] = s128.astype(np.float32)
    st[1, 64:] = np.pi / 2          # cos rows: phase shift folded into the matmul

    shared = {
        "st": st,
        "w1t": w1p.reshape(3, 128, 512),
        "w2t": w2t.reshape(4, 128, 256),
        "lwb": np.ascontiguousarray(lwb),
        "b1c": b1c,
    }

    in_maps = []
    for core in range(NCORES):
        g = core // 2
        b0 = 4 * (core % 2)
        # [4b, N, 3] -> [3, NPTS] -> [3, NT, T] -> [NT, 3, T], plus a ones row
        pcc = pcs[g, b0 : b0 + 4].reshape(NPTS, 3).T
        pcc = np.ascontiguousarray(pcc.reshape(3, NT, T).transpose(1, 0, 2))
        xsc = np.empty((NT, 2, 3, T), np.float32)
        xsc[:, 0] = pcc
        xsc[:, 1] = 1.0
        ohc = onehot[b0 : b0 + 4].reshape(NPTS, 10).T
        ohc = np.ascontiguousarray(ohc.reshape(10, NT, T).transpose(1, 0, 2))
        in_maps.append({"xs": xsc, "oh": ohc, **shared})
    return in_maps


def _get_nc():
    if "nc" not in _CACHE:
        _CACHE["nc"] = _build_program()
    return _CACHE["nc"]


def _run_device(in_maps, trace=False, **kw):
    nc = _get_nc()
    return run_bass_kernel_spmd(nc, in_maps, list(range(NCORES)), trace=trace, **kw)


def kernel(point_coord, labels, pc_range, noise, query_pos, label_weight, W1, b1, W2, b2):
    in_maps = _host_prep(
        point_coord, labels, pc_range, noise, label_weight, W1, b1, W2, b2
    )
    res = _run_device(in_maps)

    qp = np.asarray(query_pos, np.float32)
    out = np.empty((G * B, N, 4 * F), np.float32)
    out[:, :, : 2 * F] = qp.reshape(G * B, N, 2 * F)
    for core in range(NCORES):
        q = res.results[core]["q"]                       # [256, NPTS]
        q = q.reshape(2 * F, BPC, N).transpose(1, 2, 0)  # [4, N, 256]
        out[4 * core : 4 * core + 4, :, 2 * F :] = q
    return out


# revision 3
# speedup vs baseline: 1.5169x; 1.5169x over previous
"""Trainium2 Bass kernel for nn_GroupPointEncoder.

Reference computation (G=4, B=8, N=2048, F=128):
  std = 2 or 4 per point by label class
  coords = [point_coord, (point_coord + noise*std)[1:]]           # [G,B,N,3]
  normed = (coords - low) / (high - low)
  pe     = interleaved sin/cos embedding, (y,x,z) order            # [G,B,N,384]
  h      = relu(pe @ W1.T + b1)                                    # [G,B,N,512]
  pos    = h @ W2.T + b2                                           # [G,B,N,256]
  query  = label_weight[labels] + pos
  out    = concat([query_pos, query], -1).reshape(G*B, N, 512)

Sharding: data-parallel over the G*B=32 (g,b) pairs, 4 per core, 8 cores.
Each core computes its 4*2048=8192 points' `query` half on device; the
query_pos half is a passthrough assembled on the host.

Device pipeline per 512-point tile, 3-tile software pipelining so the PE
(the pole at ~5.4us/tile) never starves and HAM stays at 2.4 GHz:
  PE  : arg[128,3,T](PSUM) = [s; phase]^T @ [xs_c; 1]   3 K=2 f32r matmuls
        hp = W1p @ pe                                   12 f32r matmuls
        qp = lwb @ onehot + W2 @ h                      10 f32r matmuls
  ACT : y = Copy(arg * 1/2pi) PSUM->SBUF;  relu x2;  pe = Sin(2pi * red)
  Pool: yr = (y + 1.5*2^23) - 1.5*2^23    (round-to-nearest, pure f32)
  DVE : relu x2 (ts add+max);  red = y - yr;  q copies PSUM->SBUF
All range reduction is float-only (gpsimd dtype converts are ~5us/op).
"""
import sys
import math

sys.path.insert(0, "/opt/trn_rl_repo")

import numpy as np
from contextlib import ExitStack

import concourse.bass as bass
import concourse.tile as tile
from concourse import bacc, library_config, mybir
from concourse.bass_utils import run_bass_kernel_spmd

# problem constants (hardcoded per contract)
G, B, N, F = 4, 8, 2048, 128
NCORES = 8
BPC = B * G // NCORES          # 4 (g,b) pairs per core
NPTS = BPC * N                 # 8192 points per core
T = 512                        # points per tile
NT = NPTS // T                 # 16 tiles
TWO_PI = 2.0 * math.pi
INV_TWO_PI = 1.0 / TWO_PI
C15 = float(3 * 2 ** 22)       # 1.5*2^23: round-to-int magic for |y| <= 2
F32 = mybir.dt.float32
F32R = mybir.dt.float32r

_CACHE = {}


def _build_program():
    nc = bacc.Bacc("TRN2", target_bir_lowering=False, debug=False, num_devices=NCORES)

    xs_d = nc.dram_tensor("xs", [NT, 2, 3, T], F32R, kind="ExternalInput").ap()
    oh_d = nc.dram_tensor("oh", [NT, 10, T], F32R, kind="ExternalInput").ap()
    st_d = nc.dram_tensor("st", [2, 128], F32R, kind="ExternalInput").ap()
    w1t_d = nc.dram_tensor("w1t", [3, 128, 512], F32R, kind="ExternalInput").ap()
    w2t_d = nc.dram_tensor("w2t", [4, 128, 256], F32R, kind="ExternalInput").ap()
    lwb_d = nc.dram_tensor("lwb", [10, 256], F32R, kind="ExternalInput").ap()
    b1c_d = nc.dram_tensor("b1c", [128, 4], F32, kind="ExternalInput").ap()
    q_d = nc.dram_tensor("q", [256, NPTS], F32, kind="ExternalOutput").ap()

    with tile.TileContext(nc) as tc, ExitStack() as ctx:
        cpool = ctx.enter_context(tc.tile_pool(name="consts", bufs=1))
        wpool = ctx.enter_context(tc.tile_pool(name="weights", bufs=1))
        xsp = ctx.enter_context(tc.tile_pool(name="xs", bufs=3))
        ohp = ctx.enter_context(tc.tile_pool(name="oh", bufs=5))
        yp = ctx.enter_context(tc.tile_pool(name="y", bufs=2))
        yrp = ctx.enter_context(tc.tile_pool(name="yr", bufs=2))
        redp = ctx.enter_context(tc.tile_pool(name="red", bufs=2))
        pep = ctx.enter_context(tc.tile_pool(name="pe", bufs=3))
        hpool = ctx.enter_context(tc.tile_pool(name="h", bufs=2))
        qsp = ctx.enter_context(tc.tile_pool(name="qs", bufs=3))
        psum_a = ctx.enter_context(tc.tile_pool(name="pa", bufs=1, space="PSUM"))
        psum_h = ctx.enter_context(tc.tile_pool(name="ph", bufs=2, space="PSUM"))
        psum_q = ctx.enter_context(tc.tile_pool(name="pq", bufs=2, space="PSUM"))

        nc.gpsimd.load_library(library_config.proxy)
        st = cpool.tile([2, 128], F32R)
        nc.sync.dma_start(st[:], st_d[:])
        b1c = cpool.tile([128, 4], F32)
        nc.sync.dma_start(b1c[:], b1c_d[:])
        lwb = cpool.tile([10, 256], F32R)
        nc.sync.dma_start(lwb[:], lwb_d[:])

        w1t = []
        for k in range(3):
            w = wpool.tile([128, 512], F32R, name=f"w1t{k}", tag=f"w1t{k}")
            nc.sync.dma_start(w[:], w1t_d[k])
            w1t.append(w)
        w2t = []
        for k in range(4):
            w = wpool.tile([128, 256], F32R, name=f"w2t{k}", tag=f"w2t{k}")
            nc.sync.dma_start(w[:], w2t_d[k])
            w2t.append(w)

        xs_t = [None] * NT
        oh_t = [None] * NT
        arg_t = [None] * NT
        y_t = [None] * NT
        red_t = [None] * NT
        pe_t = [None] * NT

        Relu = mybir.ActivationFunctionType.Relu
        Copy = mybir.ActivationFunctionType.Copy
        Sin = mybir.ActivationFunctionType.Sin

        for i in range(NT + 3):
            t, r, s, u = i, i - 1, i - 2, i - 3

            if t < NT:
                xs = xsp.tile([2, 3, T], F32R, tag="xs")
                nc.sync.dma_start(xs[:], xs_d[t])
                oh = ohp.tile([10, T], F32R, tag="oh")
                nc.sync.dma_start(oh[:], oh_d[t])
                xs_t[t], oh_t[t] = xs, oh

            if 0 <= r < NT:
                # y = arg / 2pi  (frees the PSUM arg banks early in the iter)
                y = yp.tile([128, 3, T], F32, tag="y")
                nc.scalar.activation(y[:], arg_t[r][:], Copy, scale=INV_TWO_PI)
                y_t[r] = y
                arg_t[r] = None

            if u >= 0:
                hp_m = []
                h = hpool.tile([128, 4, T], F32R, tag="h")
                pe = pe_t[u]
                for m in range(4):
                    hp = psum_h.tile([128, T], F32, tag="hp")
                    hp_m.append(hp)
                    for k in range(3):
                        nc.tensor.matmul(
                            hp[:], w1t[k][:, m * 128 : (m + 1) * 128], pe[:, k, :],
                            start=(k == 0), stop=(k == 2),
                        )
                    if m < 2:
                        nc.scalar.activation(
                            h[:, m, :], hp[:], Relu, bias=b1c[:, m : m + 1]
                        )
                    else:
                        nc.vector.tensor_scalar(
                            h[:, m, :], hp[:], b1c[:, m : m + 1], 0.0,
                            op0=mybir.AluOpType.add, op1=mybir.AluOpType.max,
                        )

            if 0 <= r < NT:
                yr = yrp.tile([128, 3, T], F32, tag="yr")
                nc.gpsimd.tensor_scalar(
                    yr[:], y_t[r][:], C15, -C15,
                    op0=mybir.AluOpType.add, op1=mybir.AluOpType.add,
                )
                red = redp.tile([128, 3, T], F32, tag="red")
                nc.vector.tensor_tensor(
                    red[:], y_t[r][:], yr[:], op=mybir.AluOpType.subtract
                )
                red_t[r] = red
                y_t[r] = None

            if 0 <= s < NT:
                peo = pep.tile([128, 3, T], F32R, tag="pe")
                nc.scalar.activation(peo[:], red_t[s][:], Sin, scale=TWO_PI)
                pe_t[s] = peo
                red_t[s] = None

            if u >= 0:
                for mp in range(2):
                    qp = psum_q.tile([128, T], F32, tag="qp")
                    nc.tensor.matmul(
                        qp[:], lwb[:, mp * 128 : (mp + 1) * 128], oh_t[u][:],
                        start=True, stop=False,
                    )
                    for k in range(4):
                        nc.tensor.matmul(
                            qp[:], w2t[k][:, mp * 128 : (mp + 1) * 128],
                            h[:, k, :], start=False, stop=(k == 3),
                        )
                    qs = qsp.tile([128, T], F32, tag="qs")
                    nc.vector.tensor_copy(qs[:], qp[:])
                    nc.sync.dma_start(
                        q_d[mp * 128 : (mp + 1) * 128, u * T : (u + 1) * T], qs[:]
                    )
                xs_t[u] = oh_t[u] = pe_t[u] = None

            if t < NT:
                # arg_c = s*x + phase, K=2 outer products at iter end (PSUM
                # bank freed by y at the start of the next iter)
                arg = psum_a.tile([128, 3, T], F32, tag="arg")
                for c in range(3):
                    nc.tensor.matmul(
                        arg[:, c, :], st[:], xs_t[t][:, c, :], start=True, stop=True
                    )
                arg_t[t] = arg

    nc.compile()
    return nc


def _host_prep(point_coord, labels, pc_range, noise, label_weight, W1, b1, W2, b2):
    """Build the per-core input maps (host-side sharding + weight prep)."""
    pc32 = np.asarray(point_coord, np.float32)
    lab = np.asarray(labels)
    noi = np.asarray(noise, np.float32)
    rng = np.asarray(pc_range, np.float32)

    small = (lab == 0) | (lab >= 6)
    std = np.where(small, 2.0, 4.0).astype(np.float32)            # [B,N]
    coords = pc32[None] + noi * std[None, :, :, None]             # [G,B,N,3]
    coords[0] = pc32                                              # group 0 originals
    low, high = rng[:3], rng[3:]
    pcs = (coords - low) / (high - low) * np.float32(TWO_PI)      # [G,B,N,3]
    pcs = pcs[..., [1, 0, 2]]   # reference concatenates pe in (y,x,z) order
    onehot = np.eye(10, dtype=np.float32)[np.asarray(lab, np.int64)]  # [B,N,10]

    # feature permutation: kernel row c*128+k -> ref feature c*128+2k (sin),
    # row c*128+64+k -> c*128+2k+1 (cos)
    perm = np.empty(3 * F, np.int64)
    for c in range(3):
        for k in range(64):
            perm[c * 128 + k] = c * 128 + 2 * k
            perm[c * 128 + 64 + k] = c * 128 + 2 * k + 1
    w1p = np.ascontiguousarray(np.asarray(W1, np.float32)[:, perm].T)  # [384,512]
    w2t = np.ascontiguousarray(np.asarray(W2, np.float32).T)           # [512,256]
    lwb = np.asarray(label_weight, np.float32) + np.asarray(b2, np.float32)[None]
    b1c = np.ascontiguousarray(np.asarray(b1, np.float32).reshape(4, 128).T)

    k64 = np.arange(64, dtype=np.float64)
    s64 = 10000.0 ** (-k64 / 64.0)
    s128 = np.concatenate([s64, s64])
    st = np.zeros((2, 128), np.float32)
    st[0] = s128.astype(np.float32)
    st[1, 64:] = np.pi / 2          # cos rows: phase shift folded into the matmul

    shared = {
        "st": st,
        "w1t": w1p.reshape(3, 128, 512),
        "w2t": w2t.reshape(4, 128, 256),
        "lwb": np.ascontiguousarray(lwb),
        "b1c": b1c,
    }

    in_maps = []
    for core in range(NCORES):
        g = core // 2
        b0 = 4 * (core % 2)
        # [4b, N, 3] -> [3, NPTS] -> [3, NT, T] -> [NT, 3, T], plus a ones row
        pcc = pcs[g, b0 : b0 + 4].reshape(NPTS, 3).T
        pcc = np.ascontiguousarray(pcc.reshape(3, NT, T).transpose(1, 0, 2))
        xsc = np.empty((NT, 2, 3, T), np.float32)
        xsc[:, 0] = pcc
        xsc[:, 1] = 1.0
        ohc = onehot[b0 : b0 + 4].reshape(NPTS, 10).T
        ohc = np.ascontiguousarray(ohc.reshape(10, NT, T).transpose(1, 0, 2))
        in_maps.append({"xs": xsc, "oh": ohc, **shared})
    return in_maps


def _get_nc():
    if "nc" not in _CACHE:
        _CACHE["nc"] = _build_program()
    return _CACHE["nc"]


def _run_device(in_maps, trace=False, **kw):
    nc = _get_nc()
    return run_bass_kernel_spmd(nc, in_maps, list(range(NCORES)), trace=trace, **kw)


def kernel(point_coord, labels, pc_range, noise, query_pos, label_weight, W1, b1, W2, b2):
    in_maps = _host_prep(
        point_coord, labels, pc_range, noise, label_weight, W1, b1, W2, b2
    )
    res = _run_device(in_maps)

    qp = np.asarray(query_pos, np.float32)
    out = np.empty((G * B, N, 4 * F), np.float32)
    out[:, :, : 2 * F] = qp.reshape(G * B, N, 2 * F)
    for core in range(NCORES):
        q = res.results[core]["q"]                       # [256, NPTS]
        q = q.reshape(2 * F, BPC, N).transpose(1, 2, 0)  # [4, N, 256]
        out[4 * core : 4 * core + 4, :, 2 * F :] = q
    return out


# revision 4
# speedup vs baseline: 4.8916x; 3.2247x over previous
"""Trainium2 Bass kernel for nn_GroupPointEncoder.

Reference computation (G=4, B=8, N=2048, F=128):
  std = 2 or 4 per point by label class
  coords = [point_coord, (point_coord + noise*std)[1:]]           # [G,B,N,3]
  normed = (coords - low) / (high - low)
  pe     = interleaved sin/cos embedding, (y,x,z) order            # [G,B,N,384]
  h      = relu(pe @ W1.T + b1)                                    # [G,B,N,512]
  pos    = h @ W2.T + b2                                           # [G,B,N,256]
  query  = label_weight[labels] + pos
  out    = concat([query_pos, query], -1).reshape(G*B, N, 512)

Sharding: data-parallel over the G*B=32 (g,b) pairs, 4 per core, 8 cores.
Each core computes its 4*2048=8192 points' `query` half on device; the
query_pos half is a passthrough assembled on the host.

Device pipeline per 512-point tile (3-tile software pipelining keeps the
PE -- the ~5.4us/tile pole -- saturated and HAM-warm at 2.4 GHz):
  PE  : argq[128,3,T](PSUM) = [s*2^18/2pi; phase_q]^T @ [xs_c; 1]  (3 K=2 MMs)
        hp = W1p @ pe        12 f32r matmuls      (phase in Q18 fixed point)
        qp = lwb @ onehot + W2 @ h                10 f32r matmuls
  DVE : kq = convert(argq -> i32, RTNE); kfrac = kq & (2^18-1);  q copies
  ACT : relu x4 (per-m bias);  pe = Sin(-2pi/2^18 * kfrac + pi)
The bitwise AND is an exact mod 2^18 (two's complement), and
sin(pi - x) = sin(x) maps the [0,1) fraction into the Sin spline's
(-pi, pi] valid domain.  No gpsimd (its elementwise ops run ~26us) and
no tensor_tensor (DVE two-tensor ops run ~11us vs ~1-2us tensor_scalar).
"""
import sys
import math

sys.path.insert(0, "/opt/trn_rl_repo")

import numpy as np
from contextlib import ExitStack

import concourse.bass as bass
import concourse.tile as tile
from concourse import bacc, library_config, mybir
from concourse.bass_utils import run_bass_kernel_spmd

# problem constants (hardcoded per contract)
G, B, N, F = 4, 8, 2048, 128
NCORES = 8
BPC = B * G // NCORES          # 4 (g,b) pairs per core
NPTS = BPC * N                 # 8192 points per core
T = 512                        # points per tile
NT = NPTS // T                 # 16 tiles
TWO_PI = 2.0 * math.pi
Q = 2 ** 18                    # fixed-point phase scale (1 turn = 2^18)
MASK = Q - 1
F32 = mybir.dt.float32
F32R = mybir.dt.float32r
I32 = mybir.dt.int32

_CACHE = {}


def _build_program():
    nc = bacc.Bacc("TRN2", target_bir_lowering=False, debug=False, num_devices=NCORES)

    xs_d = nc.dram_tensor("xs", [NT, 2, 3, T], F32R, kind="ExternalInput").ap()
    oh_d = nc.dram_tensor("oh", [NT, 10, T], F32R, kind="ExternalInput").ap()
    st_d = nc.dram_tensor("st", [2, 128], F32R, kind="ExternalInput").ap()
    bv_d = nc.dram_tensor("bv", [128, 1], F32, kind="ExternalInput").ap()
    w1t_d = nc.dram_tensor("w1t", [3, 128, 512], F32R, kind="ExternalInput").ap()
    w2t_d = nc.dram_tensor("w2t", [4, 128, 256], F32R, kind="ExternalInput").ap()
    lwb_d = nc.dram_tensor("lwb", [10, 256], F32R, kind="ExternalInput").ap()
    b1c_d = nc.dram_tensor("b1c", [128, 4], F32, kind="ExternalInput").ap()
    q_d = nc.dram_tensor("q", [256, NPTS], F32, kind="ExternalOutput").ap()

    with tile.TileContext(nc) as tc, ExitStack() as ctx:
        cpool = ctx.enter_context(tc.tile_pool(name="consts", bufs=1))
        wpool = ctx.enter_context(tc.tile_pool(name="weights", bufs=1))
        xsp = ctx.enter_context(tc.tile_pool(name="xs", bufs=3))
        ohp = ctx.enter_context(tc.tile_pool(name="oh", bufs=5))
        kqp = ctx.enter_context(tc.tile_pool(name="kq", bufs=2))
        kfp = ctx.enter_context(tc.tile_pool(name="kf", bufs=3))
        pep = ctx.enter_context(tc.tile_pool(name="pe", bufs=3))
        hpool = ctx.enter_context(tc.tile_pool(name="h", bufs=2))
        qsp = ctx.enter_context(tc.tile_pool(name="qs", bufs=3))
        psum_a = ctx.enter_context(tc.tile_pool(name="pa", bufs=1, space="PSUM"))
        psum_h = ctx.enter_context(tc.tile_pool(name="ph", bufs=2, space="PSUM"))
        psum_q = ctx.enter_context(tc.tile_pool(name="pq", bufs=2, space="PSUM"))

        st = cpool.tile([2, 128], F32R)
        nc.sync.dma_start(st[:], st_d[:])
        bv = cpool.tile([128, 1], F32)
        nc.sync.dma_start(bv[:], bv_d[:])
        b1c = cpool.tile([128, 4], F32)
        nc.sync.dma_start(b1c[:], b1c_d[:])
        lwb = cpool.tile([10, 256], F32R)
        nc.sync.dma_start(lwb[:], lwb_d[:])

        w1t = []
        for k in range(3):
            w = wpool.tile([128, 512], F32R, name=f"w1t{k}", tag=f"w1t{k}")
            nc.sync.dma_start(w[:], w1t_d[k])
            w1t.append(w)
        w2t = []
        for k in range(4):
            w = wpool.tile([128, 256], F32R, name=f"w2t{k}", tag=f"w2t{k}")
            nc.sync.dma_start(w[:], w2t_d[k])
            w2t.append(w)

        xs_t = [None] * NT
        oh_t = [None] * NT
        arg_t = [None] * NT
        kf_t = [None] * NT
        pe_t = [None] * NT

        Relu = mybir.ActivationFunctionType.Relu
        Sin = mybir.ActivationFunctionType.Sin

        for i in range(NT + 3):
            t, r, s, u = i, i - 1, i - 2, i - 3

            if t < NT:
                xs = xsp.tile([2, 3, T], F32R, tag="xs")
                nc.sync.dma_start(xs[:], xs_d[t])
                oh = ohp.tile([10, T], F32R, tag="oh")
                nc.sync.dma_start(oh[:], oh_d[t])
                xs_t[t], oh_t[t] = xs, oh

            if 0 <= r < NT:
                # kq first on DVE: frees the argq PSUM banks for this iter's
                # trailing arg matmuls
                kq = kqp.tile([128, 3, T], I32, tag="kq")
                nc.vector.tensor_copy(kq[:], arg_t[r][:])
                arg_t[r] = None
                kf = kfp.tile([128, 3, T], I32, tag="kf")
                nc.vector.tensor_scalar(
                    kf[:], kq[:], MASK, None, op0=mybir.AluOpType.bitwise_and
                )
                kf_t[r] = kf

            if u >= 0:
                h = hpool.tile([128, 4, T], F32R, tag="h")
                pe = pe_t[u]
                for m in range(4):
                    hp = psum_h.tile([128, T], F32, tag="hp")
                    for k in range(3):
                        nc.tensor.matmul(
                            hp[:], w1t[k][:, m * 128 : (m + 1) * 128], pe[:, k, :],
                            start=(k == 0), stop=(k == 2),
                        )
                    nc.scalar.activation(
                        h[:, m, :], hp[:], Relu, bias=b1c[:, m : m + 1]
                    )

            if 0 <= s < NT:
                peo = pep.tile([128, 3, T], F32R, tag="pe")
                nc.scalar.activation(
                    peo[:], kf_t[s][:], Sin, scale=-TWO_PI / Q, bias=bv[:]
                )
                pe_t[s] = peo
                kf_t[s] = None

            if u >= 0:
                for mp in range(2):
                    qp = psum_q.tile([128, T], F32, tag="qp")
                    nc.tensor.matmul(
                        qp[:], lwb[:, mp * 128 : (mp + 1) * 128], oh_t[u][:],
                        start=True, stop=False,
                    )
                    for k in range(4):
                        nc.tensor.matmul(
                            qp[:], w2t[k][:, mp * 128 : (mp + 1) * 128],
                            h[:, k, :], start=False, stop=(k == 3),
                        )
                    qs = qsp.tile([128, T], F32, tag="qs")
                    nc.vector.tensor_copy(qs[:], qp[:])
                    nc.sync.dma_start(
                        q_d[mp * 128 : (mp + 1) * 128, u * T : (u + 1) * T], qs[:]
                    )
                xs_t[u] = oh_t[u] = pe_t[u] = None

            if t < NT:
                # argq_c = (s*x + phase) * 2^18/2pi, K=2 outer products at
                # iter end (PSUM banks freed by kq at the next iter's start)
                arg = psum_a.tile([128, 3, T], F32, tag="arg")
                for c in range(3):
                    nc.tensor.matmul(
                        arg[:, c, :], st[:], xs_t[t][:, c, :], start=True, stop=True
                    )
                arg_t[t] = arg

    nc.compile()
    return nc


def _host_prep(point_coord, labels, pc_range, noise, label_weight, W1, b1, W2, b2):
    """Build the per-core input maps (host-side sharding + weight prep)."""
    pc32 = np.asarray(point_coord, np.float32)
    lab = np.asarray(labels)
    noi = np.asarray(noise, np.float32)
    rng = np.asarray(pc_range, np.float32)

    small = (lab == 0) | (lab >= 6)
    std = np.where(small, 2.0, 4.0).astype(np.float32)            # [B,N]
    coords = pc32[None] + noi * std[None, :, :, None]             # [G,B,N,3]
    coords[0] = pc32                                              # group 0 originals
    low, high = rng[:3], rng[3:]
    pcs = (coords - low) / (high - low) * np.float32(TWO_PI)      # [G,B,N,3]
    pcs = pcs[..., [1, 0, 2]]   # reference concatenates pe in (y,x,z) order
    onehot = np.eye(10, dtype=np.float32)[np.asarray(lab, np.int64)]  # [B,N,10]

    # feature permutation: kernel row c*128+k -> ref feature c*128+2k (sin),
    # row c*128+64+k -> c*128+2k+1 (cos)
    perm = np.empty(3 * F, np.int64)
    for c in range(3):
        for k in range(64):
            perm[c * 128 + k] = c * 128 + 2 * k
            perm[c * 128 + 64 + k] = c * 128 + 2 * k + 1
    w1p = np.ascontiguousarray(np.asarray(W1, np.float32)[:, perm].T)  # [384,512]
    w2t = np.ascontiguousarray(np.asarray(W2, np.float32).T)           # [512,256]
    lwb = np.asarray(label_weight, np.float32) + np.asarray(b2, np.float32)[None]
    b1c = np.ascontiguousarray(np.asarray(b1, np.float32).reshape(4, 128).T)

    k64 = np.arange(64, dtype=np.float64)
    s64 = 10000.0 ** (-k64 / 64.0)
    s128 = np.concatenate([s64, s64])
    st = np.zeros((2, 128), np.float32)
    st[0] = (s128 * Q / (2 * np.pi)).astype(np.float32)  # phase in Q18 turns
    st[1, 64:] = float(Q // 4)      # cos rows: +pi/2 phase = 2^18/4
    bv = np.full((128, 1), np.pi, np.float32)

    shared = {
        "st": st,
        "bv": bv,
        "w1t": w1p.reshape(3, 128, 512),
        "w2t": w2t.reshape(4, 128, 256),
        "lwb": np.ascontiguousarray(lwb),
        "b1c": b1c,
    }

    in_maps = []
    for core in range(NCORES):
        g = core // 2
        b0 = 4 * (core % 2)
        # [4b, N, 3] -> [3, NPTS] -> [3, NT, T] -> [NT, 3, T], plus a ones row
        pcc = pcs[g, b0 : b0 + 4].reshape(NPTS, 3).T
        pcc = np.ascontiguousarray(pcc.reshape(3, NT, T).transpose(1, 0, 2))
        xsc = np.empty((NT, 2, 3, T), np.float32)
        xsc[:, 0] = pcc
        xsc[:, 1] = 1.0
        ohc = onehot[b0 : b0 + 4].reshape(NPTS, 10).T
        ohc = np.ascontiguousarray(ohc.reshape(10, NT, T).transpose(1, 0, 2))
        in_maps.append({"xs": xsc, "oh": ohc, **shared})
    return in_maps


def _get_nc():
    if "nc" not in _CACHE:
        _CACHE["nc"] = _build_program()
    return _CACHE["nc"]


def _run_device(in_maps, trace=False, **kw):
    nc = _get_nc()
    return run_bass_kernel_spmd(nc, in_maps, list(range(NCORES)), trace=trace, **kw)


def kernel(point_coord, labels, pc_range, noise, query_pos, label_weight, W1, b1, W2, b2):
    in_maps = _host_prep(
        point_coord, labels, pc_range, noise, label_weight, W1, b1, W2, b2
    )
    res = _run_device(in_maps)

    qp = np.asarray(query_pos, np.float32)
    out = np.empty((G * B, N, 4 * F), np.float32)
    out[:, :, : 2 * F] = qp.reshape(G * B, N, 2 * F)
    for core in range(NCORES):
        q = res.results[core]["q"]                       # [256, NPTS]
        q = q.reshape(2 * F, BPC, N).transpose(1, 2, 0)  # [4, N, 256]
        out[4 * core : 4 * core + 4, :, 2 * F :] = q
    return out


# revision 5
# speedup vs baseline: 6.4195x; 1.3124x over previous
"""Trainium2 Bass kernel for nn_GroupPointEncoder.

Reference computation (G=4, B=8, N=2048, F=128):
  std = 2 or 4 per point by label class
  coords = [point_coord, (point_coord + noise*std)[1:]]           # [G,B,N,3]
  normed = (coords - low) / (high - low)
  pe     = interleaved sin/cos embedding, (y,x,z) order            # [G,B,N,384]
  h      = relu(pe @ W1.T + b1)                                    # [G,B,N,512]
  pos    = h @ W2.T + b2                                           # [G,B,N,256]
  query  = label_weight[labels] + pos
  out    = concat([query_pos, query], -1).reshape(G*B, N, 512)

Sharding: data-parallel over the G*B=32 (g,b) pairs, 4 per core, 8 cores.
Each core computes its 4*2048=8192 points' `query` half on device; the
query_pos half is a passthrough assembled on the host.

Device pipeline per 512-point tile (3-tile software pipelining keeps the
PE -- the ~5.4us/tile pole -- saturated and HAM-warm at 2.4 GHz):
  PE  : argq[128,3,T](PSUM) = [s*2^18/2pi; phase_q]^T @ [xs_c; 1]  (3 K=2 MMs)
        hp = W1p @ pe        12 f32r matmuls      (phase in Q18 fixed point)
        qp = lwb @ onehot + W2 @ h                10 f32r matmuls
  DVE : kq = convert(argq -> i32, RTNE); kfrac = kq & (2^18-1);  q copies
  ACT : relu x4 (per-m bias);  pe = Sin(-2pi/2^18 * kfrac + pi)
The bitwise AND is an exact mod 2^18 (two's complement), and
sin(pi - x) = sin(x) maps the [0,1) fraction into the Sin spline's
(-pi, pi] valid domain.  No gpsimd (its elementwise ops run ~26us) and
no tensor_tensor (DVE two-tensor ops run ~11us vs ~1-2us tensor_scalar).
"""
import sys
import math

sys.path.insert(0, "/opt/trn_rl_repo")

import numpy as np
from contextlib import ExitStack

import concourse.bass as bass
import concourse.tile as tile
from concourse import bacc, library_config, mybir
from concourse.bass_utils import run_bass_kernel_spmd

# problem constants (hardcoded per contract)
G, B, N, F = 4, 8, 2048, 128
NCORES = 8
BPC = B * G // NCORES          # 4 (g,b) pairs per core
NPTS = BPC * N                 # 8192 points per core
T = 512                        # points per tile
NT = NPTS // T                 # 16 tiles
TWO_PI = 2.0 * math.pi
Q = 2 ** 18                    # fixed-point phase scale (1 turn = 2^18)
MASK = Q - 1
F32 = mybir.dt.float32
F32R = mybir.dt.float32r
I32 = mybir.dt.int32
BF16 = mybir.dt.bfloat16
NPBF16 = mybir.dt.np(mybir.dt.bfloat16)

_CACHE = {}


def _build_program():
    nc = bacc.Bacc("TRN2", target_bir_lowering=False, debug=False, num_devices=NCORES)

    xs_d = nc.dram_tensor("xs", [NT, 2, 3, T], F32R, kind="ExternalInput").ap()
    oh_d = nc.dram_tensor("oh", [NT, 10, T], BF16, kind="ExternalInput").ap()
    st_d = nc.dram_tensor("st", [2, 128], F32R, kind="ExternalInput").ap()
    bv_d = nc.dram_tensor("bv", [128, 1], F32, kind="ExternalInput").ap()
    w1t_d = nc.dram_tensor("w1t", [3, 128, 512], BF16, kind="ExternalInput").ap()
    w2t_d = nc.dram_tensor("w2t", [4, 128, 256], BF16, kind="ExternalInput").ap()
    lwb_d = nc.dram_tensor("lwb", [10, 256], BF16, kind="ExternalInput").ap()
    b1c_d = nc.dram_tensor("b1c", [128, 4], F32, kind="ExternalInput").ap()
    q_d = nc.dram_tensor("q", [256, NPTS], F32, kind="ExternalOutput").ap()

    with tile.TileContext(nc) as tc, ExitStack() as ctx:
        cpool = ctx.enter_context(tc.tile_pool(name="consts", bufs=1))
        wpool = ctx.enter_context(tc.tile_pool(name="weights", bufs=1))
        xsp = ctx.enter_context(tc.tile_pool(name="xs", bufs=3))
        ohp = ctx.enter_context(tc.tile_pool(name="oh", bufs=5))
        kqp = ctx.enter_context(tc.tile_pool(name="kq", bufs=2))
        kfp = ctx.enter_context(tc.tile_pool(name="kf", bufs=3))
        pep = ctx.enter_context(tc.tile_pool(name="pe", bufs=3))
        hpool = ctx.enter_context(tc.tile_pool(name="h", bufs=2))
        qsp = ctx.enter_context(tc.tile_pool(name="qs", bufs=3))
        psum_a = ctx.enter_context(tc.tile_pool(name="pa", bufs=1, space="PSUM"))
        psum_h = ctx.enter_context(tc.tile_pool(name="ph", bufs=2, space="PSUM"))
        psum_q = ctx.enter_context(tc.tile_pool(name="pq", bufs=2, space="PSUM"))

        st = cpool.tile([2, 128], F32R)
        nc.sync.dma_start(st[:], st_d[:])
        bv = cpool.tile([128, 1], F32)
        nc.sync.dma_start(bv[:], bv_d[:])
        b1c = cpool.tile([128, 4], F32)
        nc.sync.dma_start(b1c[:], b1c_d[:])
        lwb = cpool.tile([10, 256], BF16)
        nc.sync.dma_start(lwb[:], lwb_d[:])

        w1t = []
        for k in range(3):
            w = wpool.tile([128, 512], BF16, name=f"w1t{k}", tag=f"w1t{k}")
            nc.sync.dma_start(w[:], w1t_d[k])
            w1t.append(w)
        w2t = []
        for k in range(4):
            w = wpool.tile([128, 256], BF16, name=f"w2t{k}", tag=f"w2t{k}")
            nc.sync.dma_start(w[:], w2t_d[k])
            w2t.append(w)

        xs_t = [None] * NT
        oh_t = [None] * NT
        arg_t = [None] * NT
        kf_t = [None] * NT
        pe_t = [None] * NT

        Relu = mybir.ActivationFunctionType.Relu
        Sin = mybir.ActivationFunctionType.Sin

        for i in range(NT + 3):
            t, r, s, u = i, i - 1, i - 2, i - 3

            if t < NT:
                xs = xsp.tile([2, 3, T], F32R, tag="xs")
                nc.sync.dma_start(xs[:], xs_d[t])
                oh = ohp.tile([10, T], BF16, tag="oh")
                nc.sync.dma_start(oh[:], oh_d[t])
                xs_t[t], oh_t[t] = xs, oh

            if 0 <= r < NT:
                # kq first on DVE: frees the argq PSUM banks for this iter's
                # trailing arg matmuls
                kq = kqp.tile([128, 3, T], I32, tag="kq")
                nc.vector.tensor_copy(kq[:], arg_t[r][:])
                arg_t[r] = None
                kf = kfp.tile([128, 3, T], I32, tag="kf")
                nc.vector.tensor_scalar(
                    kf[:], kq[:], MASK, None, op0=mybir.AluOpType.bitwise_and
                )
                kf_t[r] = kf

            if u >= 0:
                h = hpool.tile([128, 4, T], BF16, tag="h")
                pe = pe_t[u]
                for m in range(4):
                    hp = psum_h.tile([128, T], F32, tag="hp")
                    for k in range(3):
                        nc.tensor.matmul(
                            hp[:], w1t[k][:, m * 128 : (m + 1) * 128], pe[:, k, :],
                            start=(k == 0), stop=(k == 2),
                        )
                    nc.scalar.activation(
                        h[:, m, :], hp[:], Relu, bias=b1c[:, m : m + 1]
                    )

            if 0 <= s < NT:
                peo = pep.tile([128, 3, T], BF16, tag="pe")
                nc.scalar.activation(
                    peo[:], kf_t[s][:], Sin, scale=-TWO_PI / Q, bias=bv[:]
                )
                pe_t[s] = peo
                kf_t[s] = None

            if u >= 0:
                for mp in range(2):
                    qp = psum_q.tile([128, T], F32, tag="qp")
                    nc.tensor.matmul(
                        qp[:], lwb[:, mp * 128 : (mp + 1) * 128], oh_t[u][:],
                        start=True, stop=False,
                    )
                    for k in range(4):
                        nc.tensor.matmul(
                            qp[:], w2t[k][:, mp * 128 : (mp + 1) * 128],
                            h[:, k, :], start=False, stop=(k == 3),
                        )
                    qs = qsp.tile([128, T], F32, tag="qs")
                    nc.vector.tensor_copy(qs[:], qp[:])
                    nc.sync.dma_start(
                        q_d[mp * 128 : (mp + 1) * 128, u * T : (u + 1) * T], qs[:]
                    )
                xs_t[u] = oh_t[u] = pe_t[u] = None

            if t < NT:
                # argq_c = (s*x + phase) * 2^18/2pi, K=2 outer products at
                # iter end (PSUM banks freed by kq at the next iter's start)
                arg = psum_a.tile([128, 3, T], F32, tag="arg")
                for c in range(3):
                    nc.tensor.matmul(
                        arg[:, c, :], st[:], xs_t[t][:, c, :], start=True, stop=True
                    )
                arg_t[t] = arg

    nc.compile()
    return nc


def _host_prep(point_coord, labels, pc_range, noise, label_weight, W1, b1, W2, b2):
    """Build the per-core input maps (host-side sharding + weight prep)."""
    pc32 = np.asarray(point_coord, np.float32)
    lab = np.asarray(labels)
    noi = np.asarray(noise, np.float32)
    rng = np.asarray(pc_range, np.float32)

    small = (lab == 0) | (lab >= 6)
    std = np.where(small, 2.0, 4.0).astype(np.float32)            # [B,N]
    coords = pc32[None] + noi * std[None, :, :, None]             # [G,B,N,3]
    coords[0] = pc32                                              # group 0 originals
    low, high = rng[:3], rng[3:]
    pcs = (coords - low) / (high - low) * np.float32(TWO_PI)      # [G,B,N,3]
    pcs = pcs[..., [1, 0, 2]]   # reference concatenates pe in (y,x,z) order
    onehot = np.eye(10, dtype=np.float32)[np.asarray(lab, np.int64)]  # [B,N,10]

    # feature permutation: kernel row c*128+k -> ref feature c*128+2k (sin),
    # row c*128+64+k -> c*128+2k+1 (cos)
    perm = np.empty(3 * F, np.int64)
    for c in range(3):
        for k in range(64):
            perm[c * 128 + k] = c * 128 + 2 * k
            perm[c * 128 + 64 + k] = c * 128 + 2 * k + 1
    w1p = np.ascontiguousarray(np.asarray(W1, np.float32)[:, perm].T)  # [384,512]
    w2t = np.ascontiguousarray(np.asarray(W2, np.float32).T)           # [512,256]
    lwb = np.asarray(label_weight, np.float32) + np.asarray(b2, np.float32)[None]
    b1c = np.ascontiguousarray(np.asarray(b1, np.float32).reshape(4, 128).T)

    k64 = np.arange(64, dtype=np.float64)
    s64 = 10000.0 ** (-k64 / 64.0)
    s128 = np.concatenate([s64, s64])
    st = np.zeros((2, 128), np.float32)
    st[0] = (s128 * Q / (2 * np.pi)).astype(np.float32)  # phase in Q18 turns
    st[1, 64:] = float(Q // 4)      # cos rows: +pi/2 phase = 2^18/4
    bv = np.full((128, 1), np.pi, np.float32)

    shared = {
        "st": st,
        "bv": bv,
        "w1t": w1p.reshape(3, 128, 512).astype(NPBF16),
        "w2t": w2t.reshape(4, 128, 256).astype(NPBF16),
        "lwb": np.ascontiguousarray(lwb).astype(NPBF16),
        "b1c": b1c,
    }

    in_maps = []
    for core in range(NCORES):
        g = core // 2
        b0 = 4 * (core % 2)
        # [4b, N, 3] -> [3, NPTS] -> [3, NT, T] -> [NT, 3, T], plus a ones row
        pcc = pcs[g, b0 : b0 + 4].reshape(NPTS, 3).T
        pcc = np.ascontiguousarray(pcc.reshape(3, NT, T).transpose(1, 0, 2))
        xsc = np.empty((NT, 2, 3, T), np.float32)
        xsc[:, 0] = pcc
        xsc[:, 1] = 1.0
        ohc = onehot[b0 : b0 + 4].reshape(NPTS, 10).T
        ohc = np.ascontiguousarray(ohc.reshape(10, NT, T).transpose(1, 0, 2))
        in_maps.append({"xs": xsc, "oh": ohc.astype(NPBF16), **shared})
    return in_maps


def _get_nc():
    if "nc" not in _CACHE:
        _CACHE["nc"] = _build_program()
    return _CACHE["nc"]


def _run_device(in_maps, trace=False, **kw):
    nc = _get_nc()
    return run_bass_kernel_spmd(nc, in_maps, list(range(NCORES)), trace=trace, **kw)


def kernel(point_coord, labels, pc_range, noise, query_pos, label_weight, W1, b1, W2, b2):
    in_maps = _host_prep(
        point_coord, labels, pc_range, noise, label_weight, W1, b1, W2, b2
    )
    res = _run_device(in_maps)

    qp = np.asarray(query_pos, np.float32)
    out = np.empty((G * B, N, 4 * F), np.float32)
    out[:, :, : 2 * F] = qp.reshape(G * B, N, 2 * F)
    for core in range(NCORES):
        q = res.results[core]["q"]                       # [256, NPTS]
        q = q.reshape(2 * F, BPC, N).transpose(1, 2, 0)  # [4, N, 256]
        out[4 * core : 4 * core + 4, :, 2 * F :] = q
    return out
